# revision 35
# baseline (speedup 1.0000x reference)
"""CaptionEmbedder kernel for Trainium2 (Bass), 8-core data-parallel.

Semantics (matching the reference):
    ent_idx  = clamp-to-49 of (caption_indices - 32000)   (oob -> 49)
    word_idx = caption_indices if < 32000 else pad_token
    out[b,l] = entities_encoded[b, ent_idx]  if caption_masks[b,l,0] == 1
               else word_embedding[word_idx]

The kernel is bounded by SWDGE descriptor generation: every gathered
row costs ~8.3ns of serial Q7 time (per-index generation; the 0.34ns
figure in hw_specs is for pattern-based SWDGE), so the design minimizes
gathered rows and starts generating as early as possible.

  * PE path -- each local batch owns one SBUF column of <=128 token
    slots filled by onehot[nr,128].T @ mini_table[nr,512] -> PSUM. The
    mini table holds the entity rows the batch actually uses (almost
    always just row 49, the clamp target) PLUS word tokens moved off
    the gather path to soak up the batch's spare slots. That offload
    drops the gather from 7 columns to ~5 (saves ~2.8us of descriptor
    generation).
  * gather path -- remaining word tokens, pooled globally, sorted by
    row, dealt round-robin across cores, gathered per column by the
    native SWDGE indirect DMA (the custom dma_gather ucode needs a
    ~9.4us mlp library load at exec time -- a net loss; multi-column
    offset APs are sim-only and corrupt on HW).

comb is self-loaded by gpsimd (its SWDGE queue is idle and starts right
after the preamble); the eo payload halves ride sync and scalar HWDGE
queues in parallel. Vector casts PSUM batches 0-3 (scalar: 4-7) to
bf16; sync stores word columns + entity A (interleaved by readiness),
scalar stores entity B.

All index math (combined-table rows, token permutation, onehots) is
host-side numpy; the host inverts the permutation on the way out.
Everything travels as bfloat16 (halves HBM traffic, rel err ~4e-3);
the host up-casts the result to float32.
"""

import os
import sys
from functools import lru_cache

import numpy as np
import ml_dtypes

for _p in ("/opt/trn_rl_repo",):
    if _p not in sys.path:
        sys.path.insert(0, _p)

# Problem shapes (hardcoded per contest contract).
V = 32000          # vocab size
B = 64             # batch
L = 200            # caption length
N_ENT = 50         # entities per batch
D = 512            # embedding dim
N_CORES = 8
B_LOC = B // N_CORES            # 8 batches per core
TOK = B_LOC * L                 # 1600 tokens per core
P = 128                         # SBUF partitions
TBL = V + B_LOC * N_ENT         # 32400 rows in combined table

BF16 = ml_dtypes.bfloat16

WARM_N = int(os.environ.get("CAPEMB_WARM", "0"))  # PE warmup matmuls
TARGET_WC = int(os.environ.get("CAPEMB_WC", "5"))  # gather columns goal
GQ = int(os.environ.get("CAPEMB_GQ", "4"))  # SWDGE queues for gathers
TARGET_VP = int(os.environ.get("CAPEMB_VP", "72"))  # last-column rows goal


@lru_cache(maxsize=8)
def _build(wc: int, vp: int, nr: int, era: int, erb: int, warm_n: int,
           gq: int):
    """wc word-gather columns (last holds vp<=128 rows); nr mini-table
    rows; era/erb = stored rows for PE column chunks b0-3 / b4-7."""
    import concourse.bacc as bacc
    import concourse.bass as bass
    from concourse import mybir

    i32 = mybir.dt.int32
    bf16 = mybir.dt.bfloat16
    f32 = mybir.dt.float32

    cols = wc + B_LOC
    half = B_LOC // 2
    OH = B_LOC * P              # onehot span in eo free dim (1024)
    EW = OH + B_LOC * D         # eo free width (5120)
    ESPLIT = OH + half * D      # eoA = onehots + minis b0-3; eoB = b4-7

    nc = bacc.Bacc("TRN2", target_bir_lowering=False, debug=False,
                   num_swdge_queues=max(1, gq))

    wcp = -(-wc // 8) * 8  # comb padded so partition rows stay 32B-aligned
    tbl_h = nc.dram_tensor("table", [TBL, D], bf16, kind="ExternalInput")
    comb_h = nc.dram_tensor("comb", [P, wcp], i32, kind="ExternalInput")
    eo_h = nc.dram_tensor("eo", [nr, EW], bf16, kind="ExternalInput")
    out_h = nc.dram_tensor("out", [P, cols, D], bf16, kind="ExternalOutput")
    tbl_ap = tbl_h.ap()
    out_ap = out_h.ap()

    comb_sb = nc.alloc_sbuf_tensor("comb_sb", [P, wcp], i32).ap()
    eo_sb = nc.alloc_sbuf_tensor("eo_sb", [nr, EW], bf16).ap()
    emb = nc.alloc_sbuf_tensor("emb", [P, cols, D], bf16).ap()
    warm = nc.alloc_sbuf_tensor("warm", [P, 640], bf16).ap()
    psum = [
        nc.alloc_psum_tensor(f"ps{b}", [P, D], f32).ap() for b in range(B_LOC)
    ]

    # word-column store chunks [0,1], [2,3], ... (2KB lines, fewer
    # issues); a partial final column gets its own chunk (fewer rows)
    w_chunks = []
    c = 0
    wfull = wc if vp == P else wc - 1
    while c < wfull:
        w = min(2, wfull - c)
        w_chunks.append((c, w, P))
        c += w
    if vp < P:
        w_chunks.append((wc - 1, 1, vp))
    n_stores = len(w_chunks) + 2  # word chunks + two PE-column chunks

    sem_c = nc.alloc_semaphore("sem_c")
    sem_e = nc.alloc_semaphore("sem_e")
    sem_e2 = nc.alloc_semaphore("sem_e2")
    sem_w = nc.alloc_semaphore("sem_w")
    sem_gs = [nc.alloc_semaphore(f"sem_g{k}") for k in range(wc)]
    sem_m = nc.alloc_semaphore("sem_m")
    sem_v = nc.alloc_semaphore("sem_v")
    sem_v2 = nc.alloc_semaphore("sem_v2")
    sem_s = nc.alloc_semaphore("sem_s")

    # Input loads issue before the block-entry barrier so their DMA
    # latency hides under it. comb (which gates the whole gather chain)
    # goes first on sync's HWDGE queue -- the fastest path to SBUF
    # (~9.4us); eoA follows on sync, eoB rides scalar's queue.
    nc.sync.dma_start(out=comb_sb, in_=comb_h.ap()[:, :]).then_inc(
        sem_c, 16
    )
    nc.sync.dma_start(
        out=eo_sb[:, 0:ESPLIT], in_=eo_h.ap()[:, 0:ESPLIT]
    ).then_inc(sem_e, 16)
    nc.scalar.dma_start(
        out=eo_sb[:, ESPLIT:EW], in_=eo_h.ap()[:, ESPLIT:EW]
    ).then_inc(sem_e2, 16)

    with nc.Block(no_gpsimd_drain=True) as block:

        @block.sync
        def _(sync):
            # word-column stores chase the gathers on sync's queue; the
            # fat entity stores drain on scalar's queue in parallel
            for (c0, w, rows) in w_chunks:
                for c in range(c0, c0 + w):
                    sync.wait_ge(sem_gs[c], 16)
                sync.dma_start(
                    out=out_ap[0:rows, c0 : c0 + w, :],
                    in_=emb[0:rows, c0 : c0 + w, :],
                ).then_inc(sem_s, 16)
            sync.wait_ge(sem_s, 16 * n_stores)

        @block.gpsimd
        def _(gpsimd):
            gpsimd.wait_ge(sem_c, 16)
            for c in range(wc):
                rows = vp if c == wc - 1 else P
                bi = gpsimd.indirect_dma_start(
                    out=emb[0:rows, c, :],
                    out_offset=None,
                    in_=tbl_ap[:, :],
                    in_offset=bass.IndirectOffsetOnAxis(
                        ap=comb_sb[0:rows, c : c + 1], axis=0
                    ),
                )
                if gq > 1:
                    # round-robin the columns over the SWDGE rings so
                    # their transfers drain in parallel (each ring
                    # dispatches ~130KB/us; one ring lags generation)
                    bi.ins.queue = f"qPoolDynamic{(c % gq) or ''}"
                bi.then_inc(sem_gs[c], 16)

        @block.tensor
        def _(tensor):
            if warm_n:
                tensor.wait_ge(sem_w, 1)
                for _i in range(warm_n):
                    tensor.matmul(
                        psum[0],
                        warm[:, 0:P],
                        warm[:, P:640],
                        start=True,
                        stop=True,
                    )
            tensor.wait_ge(sem_e, 16)
            for b in range(B_LOC):
                if b == half:
                    tensor.wait_ge(sem_e2, 16)
                tensor.matmul(
                    psum[b],
                    eo_sb[:, b * P : (b + 1) * P],
                    eo_sb[:, OH + b * D : OH + (b + 1) * D],
                    start=True,
                    stop=True,
                ).then_inc(sem_m, 1)

        @block.vector
        def _(vector):
            if warm_n:
                vector.memset(warm, 0).then_inc(sem_w, 1)
            for b in range(half):
                vector.wait_ge(sem_m, b + 1)
                cp = vector.tensor_copy(emb[:, wc + b, :], psum[b])
            cp.then_inc(sem_v, 1)

        @block.scalar
        def _(scalar):
            for b in range(half, B_LOC):
                scalar.wait_ge(sem_m, b + 1)
                cp = scalar.copy(emb[:, wc + b, :], psum[b])
            # the Activation sequencer can fire a DMA while a cast is
            # still in the pipe -- self-sem forces completion order
            cp.then_inc(sem_v2, 1)
            scalar.wait_ge(sem_v2, 1)
            scalar.dma_start(
                out=out_ap[0:erb, wc + half : cols, :],
                in_=emb[0:erb, wc + half : cols, :],
            ).then_inc(sem_s, 16)
            scalar.wait_ge(sem_v, 1)
            scalar.dma_start(
                out=out_ap[0:era, wc : wc + half, :],
                in_=emb[0:era, wc : wc + half, :],
            ).then_inc(sem_s, 16)

    # Block exit emitted an all-engine barrier; reset our semaphores so
    # the NEFF is re-executable (one range-clear: ids are contiguous).
    all_sems = [sem_c, sem_e, sem_e2, sem_w, *sem_gs, sem_m, sem_v, sem_v2,
                sem_s]
    nums = sorted(s.num for s in all_sems)
    assert nums == list(range(nums[0], nums[0] + len(nums)))
    nc.gpsimd.sem_clear(range(nums[0], nums[-1] + 1))

    nc.compile()
    return nc


def _shard_inputs(caption_indices, entities_encoded, word_embedding,
                  pad_token, caption_masks):
    """Returns (wc, vp, nr, era, erb, in_maps, gt_list, pe_list).

    Word tokens are core-agnostic (the word table is replicated): after
    the PE-offload skims off enough tokens to hit TARGET_WC gather
    columns, the remainder is pooled globally, sorted by row for HBM
    locality, and dealt round-robin so every core gathers the same
    count (exec time is the max over cores). PE-path tokens (entity
    tokens, moved word tokens, spill handling) stay on their home core.
    gt_list[i] maps global token ids to this core's gather slots."""
    caption_indices = np.asarray(caption_indices, dtype=np.int32)
    caption_masks = np.asarray(caption_masks, dtype=np.int32)
    word_bf = np.asarray(word_embedding, dtype=np.float32).astype(BF16)
    ent_bf = np.asarray(entities_encoded, dtype=np.float32).astype(BF16)

    # Fused combined-table row index, computed exactly as the reference.
    idx = caption_indices                      # [B, L]
    msk = caption_masks[:, :, 0]               # [B, L]
    ent_i = np.where((idx - V < 0) | (idx - V >= N_ENT), N_ENT - 1, idx - V)
    word_i = np.where(idx >= V, np.int32(pad_token), idx)

    msk_flat = msk.reshape(-1)
    wrows_flat = word_i.reshape(-1)

    # Pass 1: per core/batch PE occupancy and word-token inventory.
    cores = []
    tot_spill = 0
    for i in range(N_CORES):
        sl = slice(i * B_LOC, (i + 1) * B_LOC)
        m = msk[sl].reshape(-1)                    # [1600] local
        erow = ent_i[sl].reshape(-1)               # entity row within batch
        tok_b = np.arange(TOK) // L                # local batch id
        ent_toks, spill, wtoks = [], [], []
        for b in range(B_LOC):
            tb = np.nonzero((m == 1) & (tok_b == b))[0]
            ent_toks.append(tb[:P])
            spill.append(tb[P:])
            tot_spill += len(tb[P:])
            wtoks.append(np.nonzero((m == 0) & (tok_b == b))[0])
        cores.append({"sl": sl, "erow": erow, "ent_toks": ent_toks,
                      "spill": spill, "wtoks": wtoks, "moved": None})

    # PE offload: move word tokens into spare PE slots until the global
    # pool fits TARGET_WC gather columns per core. Spill rows join the
    # gather, so budget for the worst core's spill.
    n_words = int((msk_flat == 0).sum())
    max_spill = max(
        len(s) for c in cores for s in c["spill"]
    ) if cores else 0
    budget = N_CORES * ((TARGET_WC - 1) * P + TARGET_VP - max_spill)
    need = max(0, n_words - budget)
    moved_flags = np.zeros(B * L, dtype=bool)
    share = -(-need // (N_CORES * B_LOC)) if need else 0
    taken = [[0] * B_LOC for _ in range(N_CORES)]
    for phase in range(2):
        for ci, c in enumerate(cores):
            for b in range(B_LOC):
                cap = min(P - len(c["ent_toks"][b]), len(c["wtoks"][b]))
                quota = min(share, cap) if phase == 0 else cap
                take = min(quota - taken[ci][b], need)
                if take > 0:
                    lo = taken[ci][b]
                    mv = c["wtoks"][b][lo : lo + take]
                    moved_flags[c["sl"].start * L + mv] = True
                    taken[ci][b] += take
                    need -= take
        if need == 0:
            break
    assert need == 0, f"PE offload infeasible, {need} tokens left"
    for ci, c in enumerate(cores):
        c["moved"] = [c["wtoks"][b][: taken[ci][b]] for b in range(B_LOC)]

    # Global word pool (minus moved), sorted by row, dealt round-robin.
    word_g = np.nonzero((msk_flat == 0) & ~moved_flags)[0].astype(np.int64)
    word_g = word_g[np.argsort(wrows_flat[word_g], kind="stable")]
    assign = [word_g[i::N_CORES] for i in range(N_CORES)]

    per_core = []
    for i, c in enumerate(cores):
        sl, erow = c["sl"], c["erow"]
        spill = (np.concatenate(c["spill"]) if c["spill"]
                 else np.empty(0, np.int64))
        spill_rows = (V + N_ENT * (spill // L) + erow[spill]).astype(np.int32)
        gt_toks = np.concatenate([assign[i], i * TOK + spill])
        gt_slots = np.arange(len(gt_toks))
        rows = np.concatenate([wrows_flat[assign[i]], spill_rows])

        # per-batch PE token list (entity tokens then moved words) and
        # mini-table rows: unique entity rows, then moved word rows
        pe_toks, mini_rows, oh_sel = [], [], []
        for b in range(B_LOC):
            tb = c["ent_toks"][b]
            mv = c["moved"][b]
            u = (np.unique(erow[tb]) if len(tb)
                 else np.empty(0, erow.dtype))
            remap = {int(r): j for j, r in enumerate(u)}
            # mini rows: (is_word, row) so eo build can index both tables
            mr = [(False, int(r)) for r in u]
            sel = [remap[int(r)] for r in erow[tb]]
            wrow_remap = {}
            for t in mv:
                r = int(wrows_flat[sl.start * L + t])
                if r not in wrow_remap:
                    wrow_remap[r] = len(mr)
                    mr.append((True, r))
                sel.append(wrow_remap[r])
            pe_toks.append(np.concatenate([tb, mv]).astype(np.int64))
            mini_rows.append(mr)
            oh_sel.append(sel)
        per_core.append(
            (sl, rows, pe_toks, mini_rows, oh_sel, (gt_toks, gt_slots))
        )

    wc = max(1, max(-(-len(r) // P) for (_, r, _, _, _, _) in per_core))
    vp = max(1, max(len(r) - (wc - 1) * P for (_, r, _, _, _, _) in per_core))
    nr = max(2, max(len(mr) for pc in per_core for mr in pc[3]))
    nr += nr % 2
    half = B_LOC // 2
    era = max(1, max(len(pc[2][b]) for pc in per_core for b in range(half)))
    erb = max(
        1, max(len(pc[2][b]) for pc in per_core for b in range(half, B_LOC))
    )

    in_maps = []
    gt_list, pe_list = [], []
    OH = B_LOC * P
    for (sl, rows, pe_toks, mini_rows, oh_sel, gt_map) in per_core:
        tbl = np.concatenate(
            [word_bf, ent_bf[sl].reshape(B_LOC * N_ENT, D)], axis=0
        )
        wcp = -(-wc // 8) * 8
        cw = np.zeros(P * wcp, dtype=np.int32)     # filler -> row 0
        cw[: len(rows)] = rows
        comb_w = np.ascontiguousarray(cw.reshape(wcp, P).T)

        # eo: [nr, 8*128 onehots | 8*512 mini tables], batch-major cols
        eo = np.zeros((nr, B_LOC * (P + D)), dtype=BF16)
        for b in range(B_LOC):
            for j, (is_word, r) in enumerate(mini_rows[b]):
                eo[j, OH + b * D : OH + (b + 1) * D] = (
                    word_bf[r] if is_word else ent_bf[sl][b][r]
                )
            if oh_sel[b]:
                eo[oh_sel[b], b * P + np.arange(len(oh_sel[b]))] = 1

        im = {
            "table": np.ascontiguousarray(tbl),
            "comb": comb_w,
            "eo": np.ascontiguousarray(eo),
        }
        in_maps.append(im)
        gt_list.append(gt_map)
        pe_list.append(pe_toks)
    return wc, vp, nr, era, erb, in_maps, gt_list, pe_list


def _decode_into(out_flat, res, wc, gt_map, pe_toks, core):
    """Scatter one core's result [P, wc+8, D] into out_flat [B*L, D]."""
    gt_toks, gt_slots = gt_map
    if len(gt_toks):
        g = (
            np.transpose(res[:, :wc, :], (1, 0, 2))
            .reshape(wc * P, D)
            .astype(np.float32)
        )
        out_flat[gt_toks] = g[gt_slots]
    for b in range(B_LOC):
        tb = pe_toks[b]
        out_flat[core * TOK + tb] = res[: len(tb), wc + b, :].astype(
            np.float32
        )


LAST_RESULTS = None  # BassKernelResults of the most recent run (for test.py)


def kernel(caption_indices, entities_encoded, word_embedding, pad_token,
           caption_masks):
    global LAST_RESULTS
    from concourse.bass_utils import run_bass_kernel_spmd

    wc, vp, nr, era, erb, in_maps, gt_list, pe_list = _shard_inputs(
        caption_indices, entities_encoded, word_embedding, int(pad_token),
        caption_masks
    )
    nc = _build(wc, vp, nr, era, erb, WARM_N, GQ)
    res = run_bass_kernel_spmd(
        nc,
        in_maps,
        list(range(N_CORES)),
        trace=bool(os.environ.get("CAPEMB_TRACE")),
    )
    LAST_RESULTS = res
    out_flat = np.empty((B * L, D), dtype=np.float32)
    for i in range(N_CORES):
        _decode_into(out_flat, res.results[i]["out"], wc, gt_list[i],
                     pe_list[i], i)
    return out_flat.reshape(B, L, D)


# revision 36
# speedup vs baseline: 1.3612x; 1.3612x over previous
"""CaptionEmbedder kernel for Trainium2 (Bass), 8-core data-parallel.

Semantics (matching the reference):
    ent_idx  = clamp-to-49 of (caption_indices - 32000)   (oob -> 49)
    word_idx = caption_indices if < 32000 else pad_token
    out[b,l] = entities_encoded[b, ent_idx]  if caption_masks[b,l,0] == 1
               else word_embedding[word_idx]

The kernel is bounded by SWDGE descriptor generation: every gathered
row costs ~8.3ns of serial Q7 time (per-index generation; the 0.34ns
figure in hw_specs is for pattern-based SWDGE), so the design minimizes
gathered rows and starts generating as early as possible.

  * PE path -- each local batch owns one SBUF column of <=128 token
    slots filled by onehot[nr,128].T @ mini_table[nr,512] -> PSUM. The
    mini table holds the entity rows the batch actually uses (almost
    always just row 49, the clamp target) PLUS word tokens moved off
    the gather path to soak up the batch's spare slots. That offload
    drops the gather from 7 columns to ~5 (saves ~2.8us of descriptor
    generation).
  * gather path -- remaining word tokens, pooled globally, sorted by
    row, dealt round-robin across cores, gathered per column by the
    native SWDGE indirect DMA (the custom dma_gather ucode needs a
    ~9.4us mlp library load at exec time -- a net loss; multi-column
    offset APs are sim-only and corrupt on HW).

comb is self-loaded by gpsimd (its SWDGE queue is idle and starts right
after the preamble); the eo payload halves ride sync and scalar HWDGE
queues in parallel. Vector casts PSUM batches 0-3 (scalar: 4-7) to
bf16; sync stores word columns + entity A (interleaved by readiness),
scalar stores entity B.

All index math (combined-table rows, token permutation, onehots) is
host-side numpy; the host inverts the permutation on the way out.
Everything travels as bfloat16 (halves HBM traffic, rel err ~4e-3);
the host up-casts the result to float32.
"""

import os
import sys
from functools import lru_cache

import numpy as np
import ml_dtypes

for _p in ("/opt/trn_rl_repo",):
    if _p not in sys.path:
        sys.path.insert(0, _p)

# Problem shapes (hardcoded per contest contract).
V = 32000          # vocab size
B = 64             # batch
L = 200            # caption length
N_ENT = 50         # entities per batch
D = 512            # embedding dim
N_CORES = 8
B_LOC = B // N_CORES            # 8 batches per core
TOK = B_LOC * L                 # 1600 tokens per core
P = 128                         # SBUF partitions
TBL = V + B_LOC * N_ENT         # 32400 rows in combined table

BF16 = ml_dtypes.bfloat16

WARM_N = int(os.environ.get("CAPEMB_WARM", "0"))  # PE warmup matmuls
TARGET_WC = int(os.environ.get("CAPEMB_WC", "5"))  # gather columns goal
GQ = int(os.environ.get("CAPEMB_GQ", "4"))  # SWDGE queues for gathers
TARGET_VP = int(os.environ.get("CAPEMB_VP", "128"))  # last-column rows goal


@lru_cache(maxsize=8)
def _build(wc: int, vp: int, nr: int, era: int, erb: int, warm_n: int,
           gq: int):
    """wc word-gather columns (last holds vp<=128 rows); nr mini-table
    rows; era/erb = stored rows for PE column chunks b0-3 / b4-7."""
    import concourse.bacc as bacc
    import concourse.bass as bass
    from concourse import mybir

    i32 = mybir.dt.int32
    bf16 = mybir.dt.bfloat16
    f32 = mybir.dt.float32

    cols = wc + B_LOC
    half = B_LOC // 2
    OH = B_LOC * P              # onehot span in eo free dim (1024)
    EW = OH + B_LOC * D         # eo free width (5120)
    ESPLIT = OH + half * D      # eoA = onehots + minis b0-3; eoB = b4-7

    nc = bacc.Bacc("TRN2", target_bir_lowering=False, debug=False,
                   num_swdge_queues=max(1, gq))

    wcp = -(-wc // 8) * 8  # comb padded so partition rows stay 32B-aligned
    tbl_h = nc.dram_tensor("table", [TBL, D], bf16, kind="ExternalInput")
    comb_h = nc.dram_tensor("comb", [P, wcp], i32, kind="ExternalInput")
    eo_h = nc.dram_tensor("eo", [nr, EW], bf16, kind="ExternalInput")
    out_h = nc.dram_tensor("out", [P, cols, D], bf16, kind="ExternalOutput")
    tbl_ap = tbl_h.ap()
    out_ap = out_h.ap()

    comb_sb = nc.alloc_sbuf_tensor("comb_sb", [P, wcp], i32).ap()
    eo_sb = nc.alloc_sbuf_tensor("eo_sb", [nr, EW], bf16).ap()
    emb = nc.alloc_sbuf_tensor("emb", [P, cols, D], bf16).ap()
    warm = nc.alloc_sbuf_tensor("warm", [P, 640], bf16).ap()
    psum = [
        nc.alloc_psum_tensor(f"ps{b}", [P, D], f32).ap() for b in range(B_LOC)
    ]

    # word-column store chunks [0,1], [2,3], ... (2KB lines, fewer
    # issues); a partial final column gets its own chunk (fewer rows)
    w_chunks = []
    c = 0
    wfull = wc if vp == P else wc - 1
    while c < wfull:
        w = min(2, wfull - c)
        w_chunks.append((c, w, P))
        c += w
    if vp < P:
        w_chunks.append((wc - 1, 1, vp))
    n_stores = len(w_chunks) + 2  # word chunks + two PE-column chunks

    sem_c = nc.alloc_semaphore("sem_c")
    sem_e = nc.alloc_semaphore("sem_e")
    sem_e2 = nc.alloc_semaphore("sem_e2")
    sem_w = nc.alloc_semaphore("sem_w")
    sem_gs = [nc.alloc_semaphore(f"sem_g{k}") for k in range(wc)]
    sem_m = nc.alloc_semaphore("sem_m")
    sem_v = nc.alloc_semaphore("sem_v")
    sem_v2 = nc.alloc_semaphore("sem_v2")
    sem_s = nc.alloc_semaphore("sem_s")

    # Input loads issue before the block-entry barrier so their DMA
    # latency hides under it. comb (which gates the whole gather chain)
    # goes first on sync's HWDGE queue -- the fastest path to SBUF
    # (~9.4us); eoA follows on sync, eoB rides scalar's queue.
    nc.sync.dma_start(out=comb_sb, in_=comb_h.ap()[:, :]).then_inc(
        sem_c, 16
    )
    nc.sync.dma_start(
        out=eo_sb[:, 0:ESPLIT], in_=eo_h.ap()[:, 0:ESPLIT]
    ).then_inc(sem_e, 16)
    nc.scalar.dma_start(
        out=eo_sb[:, ESPLIT:EW], in_=eo_h.ap()[:, ESPLIT:EW]
    ).then_inc(sem_e2, 16)

    with nc.Block(no_gpsimd_drain=True) as block:

        @block.sync
        def _(sync):
            # word-column stores chase the gathers on sync's queue; the
            # fat entity stores drain on scalar's queue in parallel
            for (c0, w, rows) in w_chunks:
                for c in range(c0, c0 + w):
                    sync.wait_ge(sem_gs[c], 16)
                sync.dma_start(
                    out=out_ap[0:rows, c0 : c0 + w, :],
                    in_=emb[0:rows, c0 : c0 + w, :],
                ).then_inc(sem_s, 16)
            sync.wait_ge(sem_s, 16 * n_stores)

        @block.gpsimd
        def _(gpsimd):
            gpsimd.wait_ge(sem_c, 16)
            for c in range(wc):
                rows = vp if c == wc - 1 else P
                bi = gpsimd.indirect_dma_start(
                    out=emb[0:rows, c, :],
                    out_offset=None,
                    in_=tbl_ap[:, :],
                    in_offset=bass.IndirectOffsetOnAxis(
                        ap=comb_sb[0:rows, c : c + 1], axis=0
                    ),
                )
                if gq > 1:
                    # round-robin the columns over the SWDGE rings so
                    # their transfers drain in parallel (each ring
                    # dispatches ~130KB/us; one ring lags generation)
                    bi.ins.queue = f"qPoolDynamic{(c % gq) or ''}"
                bi.then_inc(sem_gs[c], 16)

        @block.tensor
        def _(tensor):
            if warm_n:
                tensor.wait_ge(sem_w, 1)
                for _i in range(warm_n):
                    tensor.matmul(
                        psum[0],
                        warm[:, 0:P],
                        warm[:, P:640],
                        start=True,
                        stop=True,
                    )
            tensor.wait_ge(sem_e, 16)
            for b in range(B_LOC):
                if b == half:
                    tensor.wait_ge(sem_e2, 16)
                tensor.matmul(
                    psum[b],
                    eo_sb[:, b * P : (b + 1) * P],
                    eo_sb[:, OH + b * D : OH + (b + 1) * D],
                    start=True,
                    stop=True,
                ).then_inc(sem_m, 1)

        @block.vector
        def _(vector):
            if warm_n:
                vector.memset(warm, 0).then_inc(sem_w, 1)
            for b in range(half):
                vector.wait_ge(sem_m, b + 1)
                cp = vector.tensor_copy(emb[:, wc + b, :], psum[b])
            cp.then_inc(sem_v, 1)

        @block.scalar
        def _(scalar):
            for b in range(half, B_LOC):
                scalar.wait_ge(sem_m, b + 1)
                cp = scalar.copy(emb[:, wc + b, :], psum[b])
            # the Activation sequencer can fire a DMA while a cast is
            # still in the pipe -- self-sem forces completion order
            cp.then_inc(sem_v2, 1)
            scalar.wait_ge(sem_v2, 1)
            scalar.dma_start(
                out=out_ap[0:erb, wc + half : cols, :],
                in_=emb[0:erb, wc + half : cols, :],
            ).then_inc(sem_s, 16)
            scalar.wait_ge(sem_v, 1)
            scalar.dma_start(
                out=out_ap[0:era, wc : wc + half, :],
                in_=emb[0:era, wc : wc + half, :],
            ).then_inc(sem_s, 16)

    # Block exit emitted an all-engine barrier; reset our semaphores so
    # the NEFF is re-executable (one range-clear: ids are contiguous).
    all_sems = [sem_c, sem_e, sem_e2, sem_w, *sem_gs, sem_m, sem_v, sem_v2,
                sem_s]
    nums = sorted(s.num for s in all_sems)
    assert nums == list(range(nums[0], nums[0] + len(nums)))
    nc.gpsimd.sem_clear(range(nums[0], nums[-1] + 1))

    nc.compile()
    return nc


def _shard_inputs(caption_indices, entities_encoded, word_embedding,
                  pad_token, caption_masks):
    """Returns (wc, vp, nr, era, erb, in_maps, gt_list, pe_list).

    Word tokens are core-agnostic (the word table is replicated): after
    the PE-offload skims off enough tokens to hit TARGET_WC gather
    columns, the remainder is pooled globally, sorted by row for HBM
    locality, and dealt round-robin so every core gathers the same
    count (exec time is the max over cores). PE-path tokens (entity
    tokens, moved word tokens, spill handling) stay on their home core.
    gt_list[i] maps global token ids to this core's gather slots."""
    caption_indices = np.asarray(caption_indices, dtype=np.int32)
    caption_masks = np.asarray(caption_masks, dtype=np.int32)
    word_bf = np.asarray(word_embedding, dtype=np.float32).astype(BF16)
    ent_bf = np.asarray(entities_encoded, dtype=np.float32).astype(BF16)

    # Fused combined-table row index, computed exactly as the reference.
    idx = caption_indices                      # [B, L]
    msk = caption_masks[:, :, 0]               # [B, L]
    ent_i = np.where((idx - V < 0) | (idx - V >= N_ENT), N_ENT - 1, idx - V)
    word_i = np.where(idx >= V, np.int32(pad_token), idx)

    msk_flat = msk.reshape(-1)
    wrows_flat = word_i.reshape(-1)

    # Pass 1: per core/batch PE occupancy and word-token inventory.
    cores = []
    tot_spill = 0
    for i in range(N_CORES):
        sl = slice(i * B_LOC, (i + 1) * B_LOC)
        m = msk[sl].reshape(-1)                    # [1600] local
        erow = ent_i[sl].reshape(-1)               # entity row within batch
        tok_b = np.arange(TOK) // L                # local batch id
        ent_toks, spill, wtoks = [], [], []
        for b in range(B_LOC):
            tb = np.nonzero((m == 1) & (tok_b == b))[0]
            ent_toks.append(tb[:P])
            spill.append(tb[P:])
            tot_spill += len(tb[P:])
            wtoks.append(np.nonzero((m == 0) & (tok_b == b))[0])
        cores.append({"sl": sl, "erow": erow, "ent_toks": ent_toks,
                      "spill": spill, "wtoks": wtoks, "moved": None})

    # PE offload: move word tokens into spare PE slots until the global
    # pool fits TARGET_WC gather columns per core. Spill rows join the
    # gather, so budget for the worst core's spill.
    n_words = int((msk_flat == 0).sum())
    max_spill = max(
        len(s) for c in cores for s in c["spill"]
    ) if cores else 0
    budget = N_CORES * ((TARGET_WC - 1) * P + TARGET_VP - max_spill)
    need = max(0, n_words - budget)
    moved_flags = np.zeros(B * L, dtype=bool)
    share = -(-need // (N_CORES * B_LOC)) if need else 0
    taken = [[0] * B_LOC for _ in range(N_CORES)]
    for phase in range(2):
        for ci, c in enumerate(cores):
            for b in range(B_LOC):
                cap = min(P - len(c["ent_toks"][b]), len(c["wtoks"][b]))
                quota = min(share, cap) if phase == 0 else cap
                take = min(quota - taken[ci][b], need)
                if take > 0:
                    lo = taken[ci][b]
                    mv = c["wtoks"][b][lo : lo + take]
                    moved_flags[c["sl"].start * L + mv] = True
                    taken[ci][b] += take
                    need -= take
        if need == 0:
            break
    assert need == 0, f"PE offload infeasible, {need} tokens left"
    for ci, c in enumerate(cores):
        c["moved"] = [c["wtoks"][b][: taken[ci][b]] for b in range(B_LOC)]

    # Global word pool (minus moved), sorted by row, dealt round-robin.
    word_g = np.nonzero((msk_flat == 0) & ~moved_flags)[0].astype(np.int64)
    word_g = word_g[np.argsort(wrows_flat[word_g], kind="stable")]
    assign = [word_g[i::N_CORES] for i in range(N_CORES)]

    per_core = []
    for i, c in enumerate(cores):
        sl, erow = c["sl"], c["erow"]
        spill = (np.concatenate(c["spill"]) if c["spill"]
                 else np.empty(0, np.int64))
        spill_rows = (V + N_ENT * (spill // L) + erow[spill]).astype(np.int32)
        gt_toks = np.concatenate([assign[i], i * TOK + spill])
        gt_slots = np.arange(len(gt_toks))
        rows = np.concatenate([wrows_flat[assign[i]], spill_rows])

        # per-batch PE token list (entity tokens then moved words) and
        # mini-table rows: unique entity rows, then moved word rows
        pe_toks, mini_rows, oh_sel = [], [], []
        for b in range(B_LOC):
            tb = c["ent_toks"][b]
            mv = c["moved"][b]
            u = (np.unique(erow[tb]) if len(tb)
                 else np.empty(0, erow.dtype))
            remap = {int(r): j for j, r in enumerate(u)}
            # mini rows: (is_word, row) so eo build can index both tables
            mr = [(False, int(r)) for r in u]
            sel = [remap[int(r)] for r in erow[tb]]
            wrow_remap = {}
            for t in mv:
                r = int(wrows_flat[sl.start * L + t])
                if r not in wrow_remap:
                    wrow_remap[r] = len(mr)
                    mr.append((True, r))
                sel.append(wrow_remap[r])
            pe_toks.append(np.concatenate([tb, mv]).astype(np.int64))
            mini_rows.append(mr)
            oh_sel.append(sel)
        per_core.append(
            (sl, rows, pe_toks, mini_rows, oh_sel, (gt_toks, gt_slots))
        )

    wc = max(1, max(-(-len(r) // P) for (_, r, _, _, _, _) in per_core))
    vp = max(1, max(len(r) - (wc - 1) * P for (_, r, _, _, _, _) in per_core))
    nr = max(2, max(len(mr) for pc in per_core for mr in pc[3]))
    nr += nr % 2
    half = B_LOC // 2
    era = max(1, max(len(pc[2][b]) for pc in per_core for b in range(half)))
    erb = max(
        1, max(len(pc[2][b]) for pc in per_core for b in range(half, B_LOC))
    )

    in_maps = []
    gt_list, pe_list = [], []
    OH = B_LOC * P
    for (sl, rows, pe_toks, mini_rows, oh_sel, gt_map) in per_core:
        tbl = np.concatenate(
            [word_bf, ent_bf[sl].reshape(B_LOC * N_ENT, D)], axis=0
        )
        wcp = -(-wc // 8) * 8
        cw = np.zeros(P * wcp, dtype=np.int32)     # filler -> row 0
        cw[: len(rows)] = rows
        comb_w = np.ascontiguousarray(cw.reshape(wcp, P).T)

        # eo: [nr, 8*128 onehots | 8*512 mini tables], batch-major cols
        eo = np.zeros((nr, B_LOC * (P + D)), dtype=BF16)
        for b in range(B_LOC):
            for j, (is_word, r) in enumerate(mini_rows[b]):
                eo[j, OH + b * D : OH + (b + 1) * D] = (
                    word_bf[r] if is_word else ent_bf[sl][b][r]
                )
            if oh_sel[b]:
                eo[oh_sel[b], b * P + np.arange(len(oh_sel[b]))] = 1

        im = {
            "table": np.ascontiguousarray(tbl),
            "comb": comb_w,
            "eo": np.ascontiguousarray(eo),
        }
        in_maps.append(im)
        gt_list.append(gt_map)
        pe_list.append(pe_toks)
    return wc, vp, nr, era, erb, in_maps, gt_list, pe_list


def _decode_into(out_flat, res, wc, gt_map, pe_toks, core):
    """Scatter one core's result [P, wc+8, D] into out_flat [B*L, D]."""
    gt_toks, gt_slots = gt_map
    if len(gt_toks):
        g = (
            np.transpose(res[:, :wc, :], (1, 0, 2))
            .reshape(wc * P, D)
            .astype(np.float32)
        )
        out_flat[gt_toks] = g[gt_slots]
    for b in range(B_LOC):
        tb = pe_toks[b]
        out_flat[core * TOK + tb] = res[: len(tb), wc + b, :].astype(
            np.float32
        )


LAST_RESULTS = None  # BassKernelResults of the most recent run (for test.py)


def kernel(caption_indices, entities_encoded, word_embedding, pad_token,
           caption_masks):
    global LAST_RESULTS
    from concourse.bass_utils import run_bass_kernel_spmd

    wc, vp, nr, era, erb, in_maps, gt_list, pe_list = _shard_inputs(
        caption_indices, entities_encoded, word_embedding, int(pad_token),
        caption_masks
    )
    nc = _build(wc, vp, nr, era, erb, WARM_N, GQ)
    res = run_bass_kernel_spmd(
        nc,
        in_maps,
        list(range(N_CORES)),
        trace=bool(os.environ.get("CAPEMB_TRACE")),
    )
    LAST_RESULTS = res
    out_flat = np.empty((B * L, D), dtype=np.float32)
    for i in range(N_CORES):
        _decode_into(out_flat, res.results[i]["out"], wc, gt_list[i],
                     pe_list[i], i)
    return out_flat.reshape(B, L, D)
